# revision 2
# baseline (speedup 1.0000x reference)
"""Distributed brute-force kNN (k-th nearest squared L2 distance) on 8 TRN2 cores.

Strategy (classic distributed kNN):
- DB rows [100000, 512] sharded 8 ways -> 12500 rows/core; queries replicated.
- Per core: normalize db rows on device, transpose via PE, f32r (tf32) matmuls
  compute dot(q, x_hat) for all (query, row) pairs, per-500-row-chunk top-8 via
  DVE max8 scanning PSUM directly, final 2-pass top-16 refine per query tile.
- Host: gather 8x[2048,16] candidates, take 10th largest dot v, and return
  ||q_hat||^2 + ||x_hat||^2 - 2*v/||q|| (with ||x_hat||^2 = 1 to fp32 rounding).

Ranking by dot(q, x_hat) is equivalent to ranking by the reference's squared
distance ||q_hat||^2 + ||x_hat||^2 - 2*dot(q_hat, x_hat): the q-terms are
constant per query and ||x_hat||^2 = 1 +- 1e-7.
"""
import sys

sys.path.insert(0, "/opt/trn_rl_repo")

import numpy as np
from contextlib import ExitStack

B = 2048        # queries
N = 100000      # db rows
D = 512         # feature dim
NCORES = 8
NS = N // NCORES        # 12500 rows per core shard
CHUNK = 500             # db rows per chunk (one PSUM bank of dots per q-tile)
NCH = NS // CHUNK       # 25 chunks
RG = 4                  # row groups per chunk (partition packing)
RPG = CHUNK // RG       # 125 rows per group
ND = D // 128           # 4 contraction slices
NQT = B // 128          # 16 query tiles
TOPC = 8                # per-chunk candidates (HW max8)
NCAND = NCH * TOPC      # 200 candidates per (query, core)

_CACHE = {}


def _build():
    import concourse.tile as tile
    from concourse import bacc, mybir

    f32 = mybir.dt.float32
    f32r = mybir.dt.float32r
    ACT = mybir.ActivationFunctionType

    nc = bacc.Bacc("TRN2", target_bir_lowering=False, debug=False)
    xqt = nc.dram_tensor("xqt", [D, B], f32, kind="ExternalInput").ap()
    xb = nc.dram_tensor("xb", [NS, D], f32, kind="ExternalInput").ap()
    eye = nc.dram_tensor("eye", [RPG, RPG], f32, kind="ExternalInput").ap()
    out = nc.dram_tensor("out", [B, 2 * TOPC], f32, kind="ExternalOutput").ap()

    with tile.TileContext(nc) as tc, ExitStack() as ctx:
        const = ctx.enter_context(tc.tile_pool(name="const", bufs=1))
        sb = ctx.enter_context(tc.tile_pool(name="sb", bufs=3))
        sbn = ctx.enter_context(tc.tile_pool(name="sbn", bufs=2))
        sbt = ctx.enter_context(tc.tile_pool(name="sbt", bufs=2))
        cpool = ctx.enter_context(tc.tile_pool(name="cand", bufs=1))
        pst = ctx.enter_context(tc.tile_pool(name="pst", bufs=3, space="PSUM"))
        pso = ctx.enter_context(tc.tile_pool(name="pso", bufs=5, space="PSUM"))

        # queries: load f32, round once to f32r (tf32) for full-rate matmul
        xqt_raw = const.tile([128, ND * B], f32, tag="xqt_raw")
        for d in range(ND):
            nc.sync.dma_start(xqt_raw[:, d * B:(d + 1) * B],
                              xqt[d * 128:(d + 1) * 128, :])
        xqt_sb = const.tile([128, ND * B], f32r, tag="xqt")
        for d in range(ND):
            nc.scalar.activation(xqt_sb[:, d * B:(d + 1) * B],
                                 xqt_raw[:, d * B:(d + 1) * B], ACT.Copy)
        eye_sb = const.tile([RPG, RPG], f32, tag="eye")
        nc.sync.dma_start(eye_sb[:], eye[:])

        # per-query-tile candidate buffers (filled chunk by chunk)
        cand = [cpool.tile([128, NCAND], f32, tag=f"cand{t}", name=f"cand{t}")
                for t in range(NQT)]

        for c in range(NCH):
            # load chunk rows (g p) d -> p g d : partition p, free (g, d)
            xbt = sb.tile([RPG, RG, D], f32, tag="xbt")
            nc.sync.dma_start(
                xbt[:], xb[c * CHUNK:(c + 1) * CHUNK, :].rearrange(
                    "(g p) d -> p g d", g=RG))

            # row norms: sumsq (ACT square+accum), sqrt, reciprocal
            scratch = sb.tile([RPG, D], f32, tag="scratch")
            ss = sb.tile([RPG, RG], f32, tag="ss")
            for g in range(RG):
                nc.scalar.activation(scratch[:], xbt[:, g, :], ACT.Square,
                                     accum_out=ss[:, g:g + 1])
            sn = sb.tile([RPG, RG], f32, tag="sn")
            nc.scalar.activation(sn[:], ss[:], ACT.Sqrt)
            si = sb.tile([RPG, RG], f32, tag="si")
            nc.vector.reciprocal(si[:], sn[:])

            # scale rows to unit norm
            xbn = sbn.tile([RPG, RG, D], f32, tag="xbn")
            for g in range(RG):
                nc.scalar.activation(xbn[:, g, :], xbt[:, g, :], ACT.Copy,
                                     scale=si[:, g:g + 1])

            # transpose to [d-slice, rows]: PE transpose, evac as f32r
            xbT = sbt.tile([128, ND, CHUNK], f32r, tag="xbT")
            for d in range(ND):
                pt = pst.tile([128, CHUNK], f32, tag="pt")
                for g in range(RG):
                    nc.tensor.transpose(
                        pt[:, g * RPG:(g + 1) * RPG],
                        xbn[:, g, d * 128:(d + 1) * 128],
                        eye_sb[:],
                    )
                nc.scalar.activation(xbT[:, d, :], pt[:], ACT.Copy)

            # dots for all query tiles; scan PSUM for per-chunk top-8
            for t in range(NQT):
                po = pso.tile([128, CHUNK], f32, tag="po")
                for d in range(ND):
                    nc.tensor.matmul(
                        po[:],
                        xqt_sb[:, d * B + t * 128:d * B + (t + 1) * 128].bitcast(f32r),
                        xbT[:, d, :],
                        start=(d == 0), stop=(d == ND - 1),
                    )
                nc.vector.max(cand[t][:, c * TOPC:(c + 1) * TOPC], po[:])

        # final refine per query tile: top-16 of the 200 candidates
        for t in range(NQT):
            res16 = sb.tile([128, 2 * TOPC], f32, tag="res16")
            nc.vector.max(res16[:, 0:TOPC], cand[t][:])
            cr = sb.tile([128, NCAND], f32, tag="cr")
            nc.vector.match_replace(cr[:], res16[:, 0:TOPC], cand[t][:], -1e30)
            nc.vector.max(res16[:, TOPC:2 * TOPC], cr[:])
            nc.sync.dma_start(out[t * 128:(t + 1) * 128, :], res16[:])

    nc.compile()
    return nc


def _get_nc():
    if "nc" not in _CACHE:
        _CACHE["nc"] = _build()
    return _CACHE["nc"]


def kernel(features, logits, db_features, k):
    from concourse.bass_utils import run_bass_kernel_spmd

    features = np.asarray(features, dtype=np.float32)
    db_features = np.asarray(db_features, dtype=np.float32)
    kk = int(k)
    assert kk == 10, f"kernel hardcodes k=10, got {kk}"
    assert features.shape == (B, D) and db_features.shape == (N, D)

    xqt_np = np.ascontiguousarray(features.T)
    eye_np = np.eye(RPG, dtype=np.float32)
    in_maps = [
        {
            "xqt": xqt_np,
            "xb": db_features[c * NS:(c + 1) * NS, :],
            "eye": eye_np,
        }
        for c in range(NCORES)
    ]

    nc = _get_nc()
    res = run_bass_kernel_spmd(nc, in_maps, core_ids=list(range(NCORES)))
    cand_all = np.concatenate([r["out"] for r in res.results], axis=1)  # [B, 128]

    # 10th largest dot(q, x_hat) per query
    v = -np.partition(-cand_all, kk - 1, axis=1)[:, kk - 1]

    # reproduce reference arithmetic for the returned value
    qn = np.linalg.norm(features, ord=2, axis=1)          # ||q||
    xqh = features / qn[:, None]
    q2 = np.sum(xqh * xqh, axis=1)                        # ~1
    kth = q2 + np.float32(1.0) - np.float32(2.0) * (v / qn)
    return kth.astype(np.float32).reshape(-1, 1)
